# revision 4
# baseline (speedup 1.0000x reference)
"""GATv2 autoencoder kernel for 8 Trainium2 NeuronCores.

Sharding: nodes split evenly across 8 cores (6250 each). Edges are
partitioned by destination core and sorted by destination (segment
layout). h = X@W1 is computed shard-wise and all-gathered so each core
can gather arbitrary source rows; attention scores, segment softmax
(max-free exp; scores are O(1) by construction) and alpha-weighted
aggregation are fused per 128-edge tile using a per-tile selection
matrix S[e, d] = exp(score_e) * (seg_e == d) and PE matmuls that
accumulate numerator and denominator in PSUM per 128-destination block.
conv3 reuses the saved exp/den and reconstructs g[src] = h2[src] @ W2^T
on the fly from a cheap all-gather of h2 [N, 30] (instead of gathering
512-wide g rows).
"""

import numpy as np

import concourse.bacc as bacc
import concourse.bass as bass
import concourse.mybir as mybir
import concourse.tile as tile
from concourse.bass_utils import run_bass_kernel_spmd
from concourse.masks import make_identity

N, E = 50000, 300000
IN, HID, OUT = 1024, 512, 30
NCORES = 8
NLOC = N // NCORES          # 6250
P = 128
NB = (NLOC + P - 1) // P    # 49 destination blocks per core
LAST_M = NLOC - (NB - 1) * P  # 106 rows in the last block
NEG = 0.2
EPS = 1e-12

f32 = mybir.dt.float32
i32 = mybir.dt.int32
AF = mybir.ActivationFunctionType
OP = mybir.AluOpType


def _shard_edges(src, dst):
    """Partition edges by dst core, sort by dst, pad to a shared tile
    schedule. Returns (T_list, per-core dict of [128, Nt] arrays)."""
    src = src.astype(np.int64)
    dst = dst.astype(np.int64)
    core = dst // NLOC
    percore = []
    counts = np.zeros((NCORES, NB), np.int64)
    for c in range(NCORES):
        m = core == c
        s_c = src[m]
        d_c = dst[m] - c * NLOC
        order = np.argsort(d_c, kind="stable")
        s_c, d_c = s_c[order], d_c[order]
        b = d_c // P
        counts[c] = np.bincount(b, minlength=NB)
        percore.append((s_c, d_c, b))
    T = np.maximum(1, np.ceil(counts.max(axis=0) / P)).astype(np.int64)
    Nt = int(T.sum())
    offs = np.zeros(NB + 1, np.int64)
    offs[1:] = np.cumsum(T)
    out = []
    for c in range(NCORES):
        s_c, d_c, b = percore[c]
        start = np.zeros(NB + 1, np.int64)
        start[1:] = np.cumsum(counts[c])
        pos_in_block = np.arange(len(s_c)) - start[b]
        flat = offs[b] * P + pos_in_block
        srcf = np.zeros(Nt * P, np.int32)
        segf = np.full(Nt * P, -1.0, np.float32)
        dstg = np.zeros(Nt * P, np.int32)
        srcf[flat] = s_c
        segf[flat] = (d_c - b * P).astype(np.float32)
        dstg[flat] = (d_c + c * NLOC).astype(np.int32)
        out.append({
            "srcg": np.ascontiguousarray(srcf.reshape(Nt, P).T),
            "dstg": np.ascontiguousarray(dstg.reshape(Nt, P).T),
            "segf": np.ascontiguousarray(segf.reshape(Nt, P).T),
        })
    return [int(t) for t in T], Nt, out


def _build_program(T_list, Nt):
    nc = bacc.Bacc("TRN2", target_bir_lowering=False, debug=False,
                   num_devices=NCORES)
    xt_t = nc.dram_tensor("xt", [IN, NLOC], f32, kind="ExternalInput")
    w1_t = nc.dram_tensor("w1", [IN, HID], f32, kind="ExternalInput")
    w2_t = nc.dram_tensor("w2", [HID, OUT], f32, kind="ExternalInput")
    w2t_t = nc.dram_tensor("w2t", [OUT, HID], f32, kind="ExternalInput")
    w4_t = nc.dram_tensor("w4", [HID, IN], f32, kind="ExternalInput")
    att_t = nc.dram_tensor("attb", [P, HID], f32, kind="ExternalInput")
    iota_t = nc.dram_tensor("iotam", [P, P], f32, kind="ExternalInput")
    src_t = nc.dram_tensor("srcg", [P, Nt], i32, kind="ExternalInput")
    dst_t = nc.dram_tensor("dstg", [P, Nt], i32, kind="ExternalInput")
    seg_t = nc.dram_tensor("segf", [P, Nt], f32, kind="ExternalInput")
    h2o_t = nc.dram_tensor("h2o", [NLOC, OUT], f32, kind="ExternalOutput")
    h4o_t = nc.dram_tensor("h4o", [NLOC, IN], f32, kind="ExternalOutput")
    hin_b = nc.dram_tensor("hin_b", [NLOC, HID], f32, kind="Internal")
    hfull = nc.dram_tensor("hfull", [N, HID], f32, kind="Internal",
                           addr_space="Shared")
    h2in_b = nc.dram_tensor("h2in_b", [NLOC, OUT], f32, kind="Internal")
    h2full = nc.dram_tensor("h2full", [N, OUT], f32, kind="Internal",
                            addr_space="Shared")
    rg = [list(range(NCORES))]

    with tile.TileContext(nc) as tc:
        with tc.tile_pool(name="const", bufs=1) as const:
            w1sb = []
            for k in range(IN // P):
                w = const.tile([P, HID], f32, name=f"w1_{k}")
                nc.sync.dma_start(out=w[:], in_=w1_t.ap()[k * P:(k + 1) * P, :])
                w1sb.append(w)
            w2sb = []
            for k in range(HID // P):
                w = const.tile([P, OUT], f32, name=f"w2_{k}")
                nc.sync.dma_start(out=w[:], in_=w2_t.ap()[k * P:(k + 1) * P, :])
                w2sb.append(w)
            w2tsb = const.tile([OUT, HID], f32, name="w2tsb")
            nc.sync.dma_start(out=w2tsb[:], in_=w2t_t.ap()[:])
            w4sb = []
            for k in range(HID // P):
                w = const.tile([P, IN], f32, name=f"w4_{k}")
                nc.sync.dma_start(out=w[:], in_=w4_t.ap()[k * P:(k + 1) * P, :])
                w4sb.append(w)
            attb = const.tile([P, HID], f32, name="attb")
            nc.sync.dma_start(out=attb[:], in_=att_t.ap()[:])
            iotam = const.tile([P, P], f32, name="iotam")
            nc.sync.dma_start(out=iotam[:], in_=iota_t.ap()[:])
            ident = const.tile([P, P], f32, name="ident")
            make_identity(nc, ident[:])
            srcs = const.tile([P, Nt], i32, name="srcs")
            nc.sync.dma_start(out=srcs[:], in_=src_t.ap()[:])
            dsts = const.tile([P, Nt], i32, name="dsts")
            nc.sync.dma_start(out=dsts[:], in_=dst_t.ap()[:])
            segs = const.tile([P, Nt], f32, name="segs")
            nc.sync.dma_start(out=segs[:], in_=seg_t.ap()[:])
            ones = const.tile([P, 1], f32, name="ones")
            nc.vector.memset(ones[:], 1.0)
            ex_all = const.tile([P, Nt], f32, name="ex_all")
            rden_all = const.tile([P, NB], f32, name="rden_all")

            # ---- stage A: h = X @ W1 (per-shard), write to hin_b ----
            with (
                tc.tile_pool(name="pa_lhs", bufs=3) as pa_lhs,
                tc.tile_pool(name="pa_ps", bufs=2, space="PSUM") as pa_ps,
                tc.tile_pool(name="pa_out", bufs=3) as pa_out,
            ):
                for b in range(NB):
                    m = P if b < NB - 1 else LAST_M
                    hps = pa_ps.tile([P, HID], f32, space="PSUM", tag="hps",
                                     name=f"hps{b}")
                    for k in range(IN // P):
                        xtt = pa_lhs.tile([P, P], f32, tag="xtt",
                                          name=f"xtt{b}_{k}")
                        nc.sync.dma_start(
                            out=xtt[:, :m],
                            in_=xt_t.ap()[k * P:(k + 1) * P, b * P:b * P + m])
                        nc.tensor.matmul(hps[:m, :], lhsT=xtt[:, :m],
                                         rhs=w1sb[k][:],
                                         start=(k == 0), stop=(k == IN // P - 1))
                    hsb = pa_out.tile([P, HID], f32, tag="hsb", name=f"hsb{b}")
                    nc.vector.tensor_copy(hsb[:m, :], hps[:m, :])
                    nc.sync.dma_start(out=hin_b.ap()[b * P:b * P + m, :],
                                      in_=hsb[:m, :])

            nc.gpsimd.collective_compute(
                "AllGather", OP.bypass, replica_groups=rg,
                ins=[hin_b.ap().opt()], outs=[hfull.ap().opt()])

            # ---- conv1 edge phase ----
            with (
                tc.tile_pool(name="c1g", bufs=4) as c1g,
                tc.tile_pool(name="c1v", bufs=2) as c1v,
                tc.tile_pool(name="c1ps", bufs=2, space="PSUM") as c1ps,
                tc.tile_pool(name="c1blk", bufs=2) as c1blk,
                tc.tile_pool(name="c1bps", bufs=2, space="PSUM") as c1bps,
            ):
                toff = 0
                for b in range(NB):
                    Tb = T_list[b]
                    m = P if b < NB - 1 else LAST_M
                    U = c1ps.tile([P, HID], f32, space="PSUM", tag="U",
                                  name=f"U{b}")
                    den = c1ps.tile([P, 1], f32, space="PSUM", tag="den",
                                    name=f"den{b}")
                    for tt in range(Tb):
                        t = toff + tt
                        msg = c1g.tile([P, HID], f32, tag="msg", name=f"mg{t}")
                        nc.gpsimd.indirect_dma_start(
                            out=msg[:], out_offset=None, in_=hfull.ap()[:],
                            in_offset=bass.IndirectOffsetOnAxis(
                                ap=srcs[:, t:t + 1], axis=0))
                        hd = c1g.tile([P, HID], f32, tag="hd", name=f"hh{t}")
                        nc.gpsimd.indirect_dma_start(
                            out=hd[:], out_offset=None, in_=hfull.ap()[:],
                            in_offset=bass.IndirectOffsetOnAxis(
                                ap=dsts[:, t:t + 1], axis=0))
                        ssum = c1v.tile([P, HID], f32, tag="ssum", name=f"ss{t}")
                        nc.vector.tensor_tensor(out=ssum[:], in0=msg[:],
                                                in1=hd[:], op=OP.add)
                        s02 = c1v.tile([P, HID], f32, tag="s02", name=f"s2{t}")
                        nc.scalar.mul(s02[:], ssum[:], NEG)
                        lr = c1v.tile([P, HID], f32, tag="lr", name=f"lr{t}")
                        nc.vector.tensor_tensor(out=lr[:], in0=ssum[:],
                                                in1=s02[:], op=OP.max)
                        scr = c1v.tile([P, HID], f32, tag="scr", name=f"sc{t}")
                        e_col = c1v.tile([P, 1], f32, tag="ecol", name=f"ec{t}")
                        nc.vector.affine_mul_reduce(
                            out=scr[:], accum_out=e_col[:], in0=lr[:],
                            in1=attb[:], scale=1.0, bias=0.0)
                        nc.scalar.activation(out=ex_all[:, t:t + 1],
                                             in_=e_col[:], func=AF.Exp)
                        S = c1v.tile([P, P], f32, tag="S", name=f"S{t}")
                        nc.vector.tensor_scalar(
                            out=S[:], in0=iotam[:], scalar1=segs[:, t:t + 1],
                            scalar2=ex_all[:, t:t + 1],
                            op0=OP.is_equal, op1=OP.mult)
                        nc.tensor.matmul(U[:], lhsT=S[:], rhs=msg[:],
                                         start=(tt == 0), stop=(tt == Tb - 1))
                        nc.tensor.matmul(den[:], lhsT=S[:], rhs=ones[:],
                                         start=(tt == 0), stop=(tt == Tb - 1))
                    # block epilogue: h1 = elu(U / den); h2 = h1 @ W2
                    dep = c1blk.tile([P, 1], f32, tag="dep", name=f"dep{b}")
                    nc.vector.tensor_scalar_add(dep[:], den[:], EPS)
                    nc.vector.reciprocal(rden_all[:, b:b + 1], dep[:])
                    h1p = c1blk.tile([P, HID], f32, tag="h1p", name=f"h1p{b}")
                    nc.vector.tensor_scalar(out=h1p[:], in0=U[:],
                                            scalar1=rden_all[:, b:b + 1],
                                            scalar2=None, op0=OP.mult)
                    mn = c1blk.tile([P, HID], f32, tag="mn", name=f"mn{b}")
                    nc.vector.tensor_scalar_min(mn[:], h1p[:], 0.0)
                    em = c1blk.tile([P, HID], f32, tag="em", name=f"em{b}")
                    nc.scalar.activation(out=em[:], in_=mn[:], func=AF.Exp)
                    r1 = c1blk.tile([P, HID], f32, tag="r1", name=f"r1{b}")
                    nc.vector.tensor_scalar(out=r1[:], in0=h1p[:], scalar1=0.0,
                                            scalar2=-1.0, op0=OP.max, op1=OP.add)
                    h1 = c1blk.tile([P, HID], f32, tag="h1", name=f"h1{b}")
                    nc.vector.tensor_tensor(out=h1[:], in0=em[:], in1=r1[:],
                                            op=OP.add)
                    h2ps = c1bps.tile([P, OUT], f32, space="PSUM", tag="h2ps",
                                      name=f"h2ps{b}", bufs=1)
                    for c4 in range(HID // P):
                        tp = c1bps.tile([P, P], f32, space="PSUM", tag="tp",
                                        name=f"tp{b}_{c4}")
                        nc.tensor.transpose(tp[:], h1[:, c4 * P:(c4 + 1) * P],
                                            ident[:])
                        h1T = c1blk.tile([P, P], f32, tag="h1T",
                                         name=f"h1T{b}_{c4}")
                        nc.vector.tensor_copy(h1T[:], tp[:])
                        nc.tensor.matmul(h2ps[:], lhsT=h1T[:], rhs=w2sb[c4][:],
                                         start=(c4 == 0),
                                         stop=(c4 == HID // P - 1))
                    h2sb = c1blk.tile([P, OUT], f32, tag="h2sb", name=f"h2sb{b}")
                    nc.vector.tensor_copy(h2sb[:], h2ps[:])
                    nc.sync.dma_start(out=h2o_t.ap()[b * P:b * P + m, :],
                                      in_=h2sb[:m, :])
                    nc.sync.dma_start(out=h2in_b.ap()[b * P:b * P + m, :],
                                      in_=h2sb[:m, :])
                    toff += Tb

            nc.gpsimd.collective_compute(
                "AllGather", OP.bypass, replica_groups=rg,
                ins=[h2in_b.ap().opt()], outs=[h2full.ap().opt()])

            # ---- conv3 edge phase ----
            with (
                tc.tile_pool(name="c3g", bufs=4) as c3g,
                tc.tile_pool(name="c3v", bufs=3) as c3v,
                tc.tile_pool(name="c3ps", bufs=2, space="PSUM") as c3ps,
                tc.tile_pool(name="c3blk", bufs=2) as c3blk,
            ):
                toff = 0
                for b in range(NB):
                    Tb = T_list[b]
                    m = P if b < NB - 1 else LAST_M
                    # V30[d, :] = sum_e ex_e * h2[src_e, :]  (aggregate the
                    # 30-wide h2 rows; @W2^T commutes with the sum)
                    V30 = c3ps.tile([P, OUT], f32, space="PSUM", tag="V30",
                                    name=f"V30{b}")
                    for tt in range(Tb):
                        t = toff + tt
                        g2 = c3g.tile([P, OUT], f32, tag="g2", name=f"g2{t}")
                        nc.gpsimd.indirect_dma_start(
                            out=g2[:], out_offset=None, in_=h2full.ap()[:],
                            in_offset=bass.IndirectOffsetOnAxis(
                                ap=srcs[:, t:t + 1], axis=0))
                        S2 = c3v.tile([P, P], f32, tag="S2", name=f"S2{t}")
                        nc.vector.tensor_scalar(
                            out=S2[:], in0=iotam[:], scalar1=segs[:, t:t + 1],
                            scalar2=ex_all[:, t:t + 1],
                            op0=OP.is_equal, op1=OP.mult)
                        nc.tensor.matmul(V30[:], lhsT=S2[:], rhs=g2[:],
                                         start=(tt == 0), stop=(tt == Tb - 1))
                    # block epilogue: h3 = elu((V30 / den) @ W2^T); h4 = h3 @ W4
                    v30n = c3blk.tile([P, OUT], f32, tag="v30n", name=f"v30n{b}")
                    nc.vector.tensor_scalar(out=v30n[:], in0=V30[:],
                                            scalar1=rden_all[:, b:b + 1],
                                            scalar2=None, op0=OP.mult)
                    v30tp = c3ps.tile([OUT, P], f32, space="PSUM", tag="tp3",
                                      name=f"v30tp{b}", bufs=2)
                    nc.tensor.transpose(v30tp[:], v30n[:], ident[:])
                    v30T = c3blk.tile([OUT, P], f32, tag="v30T", name=f"v30T{b}")
                    nc.vector.tensor_copy(v30T[:], v30tp[:])
                    g3ps = c3ps.tile([P, HID], f32, space="PSUM", tag="g3ps",
                                     name=f"g3ps{b}", bufs=2)
                    nc.tensor.matmul(g3ps[:], lhsT=v30T[:], rhs=w2tsb[:],
                                     start=True, stop=True)
                    h3p = c3blk.tile([P, HID], f32, tag="h3p", name=f"h3p{b}")
                    nc.vector.tensor_copy(h3p[:], g3ps[:])
                    mn3 = c3blk.tile([P, HID], f32, tag="mn3", name=f"mn3{b}")
                    nc.vector.tensor_scalar_min(mn3[:], h3p[:], 0.0)
                    em3 = c3blk.tile([P, HID], f32, tag="em3", name=f"em3{b}")
                    nc.scalar.activation(out=em3[:], in_=mn3[:], func=AF.Exp)
                    r13 = c3blk.tile([P, HID], f32, tag="r13", name=f"r13{b}")
                    nc.vector.tensor_scalar(out=r13[:], in0=h3p[:], scalar1=0.0,
                                            scalar2=-1.0, op0=OP.max,
                                            op1=OP.add)
                    h3 = c3blk.tile([P, HID], f32, tag="h3", name=f"h3{b}")
                    nc.vector.tensor_tensor(out=h3[:], in0=em3[:], in1=r13[:],
                                            op=OP.add)
                    h4ps = [
                        c3ps.tile([P, HID], f32, space="PSUM", tag=f"h4ps{nh}",
                                  name=f"h4ps{b}_{nh}", bufs=1)
                        for nh in range(IN // HID)
                    ]
                    for c4 in range(HID // P):
                        tp3 = c3ps.tile([P, P], f32, space="PSUM", tag="tp3",
                                        name=f"tp3{b}_{c4}", bufs=2)
                        nc.tensor.transpose(tp3[:], h3[:, c4 * P:(c4 + 1) * P],
                                            ident[:])
                        h3T = c3blk.tile([P, P], f32, tag="h3T",
                                         name=f"h3T{b}_{c4}")
                        nc.vector.tensor_copy(h3T[:], tp3[:])
                        for nh in range(IN // HID):
                            nc.tensor.matmul(
                                h4ps[nh][:], lhsT=h3T[:],
                                rhs=w4sb[c4][:, nh * HID:(nh + 1) * HID],
                                start=(c4 == 0), stop=(c4 == HID // P - 1))
                    h4sb = c3blk.tile([P, IN], f32, tag="h4sb", name=f"h4sb{b}")
                    for nh in range(IN // HID):
                        nc.vector.tensor_copy(
                            h4sb[:, nh * HID:(nh + 1) * HID], h4ps[nh][:])
                    nc.sync.dma_start(out=h4o_t.ap()[b * P:b * P + m, :],
                                      in_=h4sb[:m, :])
                    toff += Tb

    nc.compile()
    return nc


def kernel(features, edge_index, W1, att1, W2, W4):
    features = np.asarray(features, np.float32)
    edge_index = np.asarray(edge_index)
    W1 = np.asarray(W1, np.float32)
    att1 = np.asarray(att1, np.float32)
    W2 = np.asarray(W2, np.float32)
    W4 = np.asarray(W4, np.float32)

    T_list, Nt, edata = _shard_edges(edge_index[0], edge_index[1])
    nc = _build_program(T_list, Nt)

    attb = np.ascontiguousarray(np.tile(att1[None, :], (P, 1)))
    iotam = np.ascontiguousarray(
        np.tile(np.arange(P, dtype=np.float32)[None, :], (P, 1)))
    w2t = np.ascontiguousarray(W2.T)
    in_maps = []
    for c in range(NCORES):
        xt = np.ascontiguousarray(features[c * NLOC:(c + 1) * NLOC].T)
        in_maps.append({
            "xt": xt, "w1": W1, "w2": W2, "w2t": w2t, "w4": W4,
            "attb": attb, "iotam": iotam,
            "srcg": edata[c]["srcg"], "dstg": edata[c]["dstg"],
            "segf": edata[c]["segf"],
        })
    global LAST_NC, LAST_IN_MAPS
    LAST_NC, LAST_IN_MAPS = nc, in_maps
    res = run_bass_kernel_spmd(nc, in_maps, core_ids=list(range(NCORES)))
    h2 = np.concatenate([res.results[c]["h2o"] for c in range(NCORES)], axis=0)
    h4 = np.concatenate([res.results[c]["h4o"] for c in range(NCORES)], axis=0)
    return h2, h4


LAST_NC = None
LAST_IN_MAPS = None
